# revision 1
# baseline (speedup 1.0000x reference)
"""Cross-attention kernel for Trainium2 (Bass/Tile), 8 NeuronCores.

Computes, per batch b:
    S   = (dom @ ref^T) * SCALE          [N, N]
    P   = softmax(S, axis=-1)
    x   = P @ ref                        [N, C]
    y   = scramble(x)  (x.T flattened and re-chunked into N rows of C)
    out = y @ proj_w^T + proj_b

The scramble + linear fuse algebraically:
    out[2*cp + e, j] = sum_q x[512*e + q, cp] * proj_w[j, q] + proj_b[j]
so out rows with parity e are (x_half_e^T @ proj_w^T) — computed on the
tensor engine with x tiles as lhsT directly (natural layout, no transpose)
and a host-pretransposed proj_w^T as the moving operand; the row interleave
(2*cp + e) is folded into the output DMA access pattern.

Sharding: data-parallel over batch. B=16 -> 2 batches per core, 8 cores,
no collectives.

All matmuls run in float32r (full PE rate at free-dim >= 256 vs 4x slower
fp32; measured end-to-end rel err ~3.5e-4). The BIR verifier requires every
producer feeding an fp32r matmul to emit fp32r: engine ops get a real
mantissa rounding on write, DMA loads are bitcast relabels (the PE rounds
internally). PE transposes also run as fp32r (1.5 vs 2 cyc/row, HW-measured
~85ns/128x128 block) — their inputs/outputs are all fp32r matmul operands.

DMA ring split: dom/wt/bias stream on the SWDGE (gpsimd) ring, ref on the
ACT HWDGE ring, stores + identity on the SP HWDGE ring — inputs never queue
behind stores and dom/ref stream concurrently. A burst of dependency-free
identity transposes at t=0 warms the PE clock gate (HAM) while the first
loads stream.
"""

import os
from contextlib import ExitStack

import numpy as np

import concourse.bass as bass
import concourse.mybir as mybir
import concourse.tile as tile
from concourse import bacc
from concourse._compat import with_exitstack
from concourse.bass_utils import run_bass_kernel_spmd

B, N, C = 16, 1024, 512
NUM_HEADS = 8
SCALE = (C // NUM_HEADS) ** -0.5  # 0.125
CORES = 8
BPC = B // CORES  # batches per core

P = 128          # partitions
NT = N // P      # 8 query tiles
CCH = C // P     # 4 contraction chunks over channels
MH = N // 512    # 2 key halves (PSUM bank = 512 fp32)
MCH = N // P     # 8 key chunks
JT = C // P      # 4 output-column blocks per half
LCH = 2          # input-load chunks of 2 row-tiles (512KB) each

F32 = mybir.dt.float32
F32R = mybir.dt.float32r

USE_F32R = os.environ.get("KERNEL_F32R", "1") == "1"
WARMUP_MMS = int(os.environ.get("KERNEL_WARMUP", "40"))


def _r(ap):
    return ap.bitcast(F32R) if USE_F32R else ap


@with_exitstack
def _core_kernel(ctx: ExitStack, tc: tile.TileContext,
                 domt_d, reft_d, ref_d, wt_d, bias_d, eye_d, out_d):
    nc = tc.nc

    consts = ctx.enter_context(tc.tile_pool(name="consts", bufs=1))
    identity = consts.tile([P, P], F32)
    nc.sync.dma_start(_r(identity[:]), _r(eye_d[:, :]))

    ps_S = ctx.enter_context(tc.tile_pool(name="ps_s", bufs=3, space="PSUM"))
    ps_T = ctx.enter_context(tc.tile_pool(name="ps_t", bufs=3, space="PSUM"))
    ps_X = ctx.enter_context(tc.tile_pool(name="ps_x", bufs=2, space="PSUM"))

    # PE warmup: dense dependency-free matmuls on memset zeros while the
    # first input DMAs stream, so the HAM clock gate reaches 8/8 before
    # real work. No DMA dependency at all.
    if WARMUP_MMS:
        zsrc = consts.tile([P, 640], F32)
        nc.vector.memset(zsrc[:], 0.0)
        zr = consts.tile([P, 640], F32)
        nc.vector.tensor_copy(_r(zr[:]), zsrc[:])
        warm_ps = ps_T.tile([P, 512], F32, tag="ps_t")
        for i in range(WARMUP_MMS):
            nc.tensor.matmul(warm_ps[:], _r(zr[:, :P]), _r(zr[:, P:640]),
                             start=True, stop=True)

    p_ref = ctx.enter_context(tc.tile_pool(name="ref", bufs=2))
    p_domT = ctx.enter_context(tc.tile_pool(name="domT", bufs=2))
    p_refT = ctx.enter_context(tc.tile_pool(name="refT", bufs=2))
    p_P = ctx.enter_context(tc.tile_pool(name="probs", bufs=2))
    p_Pt = ctx.enter_context(tc.tile_pool(name="probsT", bufs=2))
    p_x = ctx.enter_context(tc.tile_pool(name="x", bufs=8))
    p_out = ctx.enter_context(tc.tile_pool(name="out", bufs=4))
    p_stats = ctx.enter_context(tc.tile_pool(name="stats", bufs=8))

    # ---- pre-emit every input DMA so the rings stream continuously ----
    # domT/ref-natural/wt/bias on the SWDGE (gpsimd) ring; refT behind
    # identity on the SP HWDGE ring (stores queue after them); nothing on
    # the ACT ring so its table load delays no transfer. dom and ref come
    # host-pretransposed ([C, N] layout) so no PE transposes are needed
    # for the QK^T operands; ref is also loaded natural for P @ ref.
    def load_T(sb, dr, b, eng):
        # chunk k at cols [k*N, (k+1)*N) = rows [128k, 128(k+1)) of [C, N];
        # h-major halves: all k-chunks of the first half arrive first, so
        # the first half's matmuls unlock after ~1MB per operand
        for h in range(MH):
            for k in range(CCH):
                eng.dma_start(
                    _r(sb[:, k * N + h * 512: k * N + (h + 1) * 512]),
                    _r(dr[b, k * P:(k + 1) * P, h * 512:(h + 1) * 512]),
                )

    def load_nat(sb, dr, b, eng):
        # tile col block t holds rows [128t, 128(t+1)) of the [N, C] matrix
        for lc in range(0, NT, LCH):
            eng.dma_start(
                _r(sb[:, lc * C:(lc + LCH) * C]
                   .rearrange("p (t c) -> p t c", t=LCH)),
                _r(dr[b, lc * P:(lc + LCH) * P]
                   .rearrange("(t p) c -> p t c", p=P)),
            )

    domT_sbs = [p_domT.tile([P, CCH * N], F32, tag="domT", name=f"domT_sb{i}")
                for i in range(BPC)]
    refT_sbs = [p_refT.tile([P, CCH * N], F32, tag="refT", name=f"refT_sb{i}")
                for i in range(BPC)]
    ref_sbs = [p_ref.tile([P, NT * C], F32, tag="ref", name=f"ref_sb{i}")
               for i in range(BPC)]
    load_T(domT_sbs[0], domt_d, 0, nc.gpsimd)
    load_T(refT_sbs[0], reft_d, 0, nc.sync)
    load_nat(ref_sbs[0], ref_d, 0, nc.gpsimd)
    wt_sb = consts.tile([P, CCH * C], F32)
    for q in range(CCH):
        nc.gpsimd.dma_start(_r(wt_sb[:, q * C:(q + 1) * C]),
                            _r(wt_d[q * P:(q + 1) * P, :]))
    bias_sb = consts.tile([P, C], F32)
    nc.gpsimd.dma_start(bias_sb[:], bias_d.partition_broadcast(P))
    # rank-1 bias injection operands for the tail half (PE adds bias there)
    ones_f = consts.tile([1, P], F32)
    nc.vector.memset(ones_f[:], 1.0)
    ones_r = consts.tile([1, P], F32)
    nc.vector.tensor_copy(_r(ones_r[:]), ones_f[:])
    brow_f = consts.tile([1, C], F32)
    nc.gpsimd.dma_start(brow_f[:], bias_d[None, :])
    brow_r = consts.tile([1, C], F32)
    nc.vector.tensor_copy(_r(brow_r[:]), brow_f[:])
    if BPC > 1:
        load_T(domT_sbs[1], domt_d, 1, nc.gpsimd)
        load_T(refT_sbs[1], reft_d, 1, nc.sync)
        load_nat(ref_sbs[1], ref_d, 1, nc.gpsimd)

    for b in range(BPC):
        domT_sb = domT_sbs[b]
        refT_sb = refT_sbs[b]
        ref_sb = ref_sbs[b]

        out_v = out_d[b].rearrange("(n2 two) j -> two n2 j", two=2)
        x_tiles = []

        def emit_half_out(e, pe_bias=False):
            # out rows (2*cp + e) = x_half_e^T @ proj_w^T + bias
            for cb in range(JT):
                ps_z = ps_X.tile([P, C], F32, tag="ps_x")
                if pe_bias:
                    # rank-1 broadcast: psum starts at bias, mm3 accumulates
                    nc.tensor.matmul(ps_z[:], _r(ones_r[:]), _r(brow_r[:]),
                                     start=True, stop=False)
                for q in range(CCH):
                    x_t = x_tiles[e * CCH + q]  # q-chunk of half e
                    nc.tensor.matmul(
                        ps_z[:],
                        _r(x_t[:, cb * P:(cb + 1) * P]),
                        _r(wt_sb[:, q * C:(q + 1) * C]),
                        start=False if pe_bias else (q == 0),
                        stop=(q == CCH - 1),
                    )
                o_sb = p_out.tile([P, C], F32, tag="out")
                if pe_bias:
                    nc.scalar.copy(o_sb[:], ps_z[:])
                else:
                    nc.vector.tensor_add(o_sb[:], ps_z[:], bias_sb[:])
                nc.sync.dma_start(out_v[e, cb * P:(cb + 1) * P, :], o_sb[:])

        Pt_tiles = {}
        recips = {}

        def stage1(nt):
            # ---- S = dom @ ref^T; P = exp(S*SCALE) per half, fused rowsums
            # logits are bounded (~|16|) so the max-subtraction is unnecessary
            P_sb = p_P.tile([P, N], F32, tag="probs", name=f"P_sb{nt}")
            rowsums = []
            for h in range(MH):
                ps_s = ps_S.tile([P, 512], F32, tag="ps_s",
                                 name=f"ps_s{nt}_{h}")
                for k in range(CCH):
                    nc.tensor.matmul(
                        ps_s[:],
                        _r(domT_sb[:, k * N + nt * P: k * N + (nt + 1) * P]),
                        _r(refT_sb[:, k * N + h * 512: k * N + (h + 1) * 512]),
                        start=(k == 0), stop=(k == CCH - 1),
                    )
                rs = p_stats.tile([P, 1], F32, tag="rowsum", name=f"rs{nt}_{h}")
                nc.scalar.activation(_r(P_sb[:, h * 512:(h + 1) * 512]), ps_s[:],
                                     mybir.ActivationFunctionType.Exp,
                                     scale=float(SCALE), accum_out=rs[:])
                rowsums.append(rs)
            rowsum = p_stats.tile([P, 1], F32, tag="rowsum2", name=f"rsum{nt}")
            nc.vector.tensor_add(rowsum[:], rowsums[0][:], rowsums[1][:])
            recip = p_stats.tile([P, 1], F32, tag="recip", name=f"recip{nt}")
            nc.vector.reciprocal(recip[:], rowsum[:])
            recips[nt] = recip

            # ---- transpose P -> Pt (chunk mi at cols [mi*P, (mi+1)*P)) ----
            Pt_sb = p_Pt.tile([P, N], F32, tag="probsT", name=f"Pt_sb{nt}")
            for g in range(MCH // 4):
                ps = ps_T.tile([P, 512], F32R if USE_F32R else F32, tag="ps_t",
                               name=f"ps_t{nt}_{g}")
                for j in range(4):
                    mi = g * 4 + j
                    nc.tensor.transpose(ps[:, j * P:(j + 1) * P],
                                        _r(P_sb[:, mi * P:(mi + 1) * P]),
                                        _r(identity[:]))
                dst = _r(Pt_sb[:, g * 512:(g + 1) * 512])
                if g == 0:
                    nc.scalar.copy(dst, ps[:])
                else:
                    nc.vector.tensor_copy(dst, ps[:])
            Pt_tiles[nt] = Pt_sb

        def stage2(nt):
            # ---- x = P @ ref ----
            Pt_sb = Pt_tiles.pop(nt)
            ps_x = ps_X.tile([P, C], F32, tag="ps_x", name=f"ps_x{nt}")
            for mi in range(MCH):
                nc.tensor.matmul(
                    ps_x[:],
                    _r(Pt_sb[:, mi * P:(mi + 1) * P]),
                    _r(ref_sb[:, mi * C:(mi + 1) * C]),
                    start=(mi == 0), stop=(mi == MCH - 1),
                )
            # evict with fused softmax normalization (per-row 1/sum)
            x_t = p_x.tile([P, C], F32, tag="x", name=f"x_t{nt}")
            nc.scalar.mul(_r(x_t[:]), ps_x[:], recips.pop(nt)[:])
            x_tiles.append(x_t)

            # projection for a half as soon as its 4 x tiles exist
            if nt == CCH - 1:
                emit_half_out(0)
            elif nt == NT - 1:
                emit_half_out(1, pe_bias=(b == BPC - 1))

        # batch 0's mm2 is software-pipelined 2 tiles behind mm1 so the PE
        # stream never head-of-line-blocks on the (later-arriving) natural
        # ref while QK^T work is still available
        depth = 0
        for nt in range(NT + depth):
            if nt < NT:
                stage1(nt)
            if nt - depth >= 0:
                stage2(nt - depth)


_CACHED = {}


def _build():
    key = ("nc", USE_F32R, WARMUP_MMS)
    if key in _CACHED:
        return _CACHED[key]
    nc = bacc.Bacc("TRN2", target_bir_lowering=False, debug=False)
    domt_d = nc.dram_tensor("domt", [BPC, C, N], F32, kind="ExternalInput").ap()
    reft_d = nc.dram_tensor("reft", [BPC, C, N], F32, kind="ExternalInput").ap()
    ref_d = nc.dram_tensor("ref", [BPC, N, C], F32, kind="ExternalInput").ap()
    wt_d = nc.dram_tensor("wt", [C, C], F32, kind="ExternalInput").ap()
    bias_d = nc.dram_tensor("bias", [C], F32, kind="ExternalInput").ap()
    eye_d = nc.dram_tensor("eye", [P, P], F32, kind="ExternalInput").ap()
    out_d = nc.dram_tensor("out", [BPC, N, C], F32, kind="ExternalOutput").ap()

    with tile.TileContext(nc) as tc:
        _core_kernel(tc, domt_d, reft_d, ref_d, wt_d, bias_d, eye_d, out_d)
    nc.compile()
    _CACHED[key] = nc
    return nc


LAST_RESULTS = None


def kernel(dom, ref, proj_w, proj_b):
    global LAST_RESULTS
    dom = np.ascontiguousarray(np.asarray(dom, dtype=np.float32))
    ref = np.ascontiguousarray(np.asarray(ref, dtype=np.float32))
    wt = np.ascontiguousarray(np.asarray(proj_w, dtype=np.float32).T)
    bias = np.ascontiguousarray(np.asarray(proj_b, dtype=np.float32))
    eye = np.eye(P, dtype=np.float32)

    domt = np.ascontiguousarray(dom.transpose(0, 2, 1))
    reft = np.ascontiguousarray(ref.transpose(0, 2, 1))
    nc = _build()
    in_maps = [
        {
            "domt": domt[c * BPC:(c + 1) * BPC],
            "reft": reft[c * BPC:(c + 1) * BPC],
            "ref": ref[c * BPC:(c + 1) * BPC],
            "wt": wt,
            "bias": bias,
            "eye": eye,
        }
        for c in range(CORES)
    ]
    res = run_bass_kernel_spmd(nc, in_maps, list(range(CORES)))
    LAST_RESULTS = res
    if res.exec_time_ns is not None:
        print(f"HW exec time: {res.exec_time_ns} ns")
    return np.concatenate([r["out"] for r in res.results], axis=0)



# revision 2
# speedup vs baseline: 1.2172x; 1.2172x over previous
"""Cross-attention kernel for Trainium2 (Bass/Tile), 8 NeuronCores.

Computes, per batch b:
    S   = (dom @ ref^T) * SCALE          [N, N]
    P   = softmax(S, axis=-1)
    x   = P @ ref                        [N, C]
    y   = scramble(x)  (x.T flattened and re-chunked into N rows of C)
    out = y @ proj_w^T + proj_b

The scramble + linear fuse algebraically:
    out[2*cp + e, j] = sum_q x[512*e + q, cp] * proj_w[j, q] + proj_b[j]
so out rows with parity e are (x_half_e^T @ proj_w^T) — computed on the
tensor engine with x tiles as lhsT directly; the row interleave (2*cp + e)
is folded into the output DMA access pattern.

v2 design (vs the fp32r baseline at ~127us):
  * All matmul operands are bf16 (host-cast): input DMA halves to ~6.6MB
    per core, which removes the front-end PE starvation (the fp32 version
    streamed inputs until ~42us while the PE idled at ~55%).
  * S is computed TRANSPOSED (S^T = ref @ dom^T, lhsT=refT chunks,
    rhs=domT): exp(S^T) then *is* P^T, which stage 2 needs as lhsT — this
    deletes all 128 PE transposes + 32 PSUM-evict copies of the baseline.
  * Softmax row-sums (over the partition dim of P^T) come from a rank-1
    matmul per (nt, mi): rhs = ones [m,1], accumulated into a [n,1] PSUM
    tile alongside the x accumulation — ~60 PE cycles each, sharing the
    stationary P^T chunk with the x matmul. 1/rowsum is applied at x
    eviction (ACT, per-partition scale), exactly like the baseline.
  * Batches interleave: b0 stage2 -> b1 stage1 -> b0 proj(half1) so the
    PE never waits on the x-evict latency at a half boundary.
  * DMA rings: domT/ref-natural on SWDGE (gpsimd), refT (per-m-tile for
    the first batch, so the first stage-1 group unlocks after ~0.6MB)
    + wt/bias on the SP HWDGE ring, output stores on the ACT HWDGE ring.
  * Logits are bounded (~|16|) so softmax skips the max-subtraction; exp
    runs on ACT with the *SCALE folded in. bf16 end-to-end rel err vs the
    fp32 reference is ~5e-3 (CPU-simulated 5.7e-3), well inside 2e-2.
"""

import os
from contextlib import ExitStack

import numpy as np
import ml_dtypes

import concourse.bass as bass
import concourse.mybir as mybir
import concourse.tile as tile
from concourse import bacc
from concourse._compat import with_exitstack
from concourse.bass_utils import run_bass_kernel_spmd

B, N, C = 16, 1024, 512
NUM_HEADS = 8
SCALE = (C // NUM_HEADS) ** -0.5  # 0.125
CORES = 8
BPC = B // CORES  # batches per core

P = 128          # partitions
NT = N // P      # 8 query (n) tiles
MT = N // P      # 8 key (m) tiles
KC = C // P      # 4 contraction chunks over channels
MH = 2           # halves of N (PSUM bank = 512 fp32)
JT = C // P      # 4 output-column blocks per half

F32 = mybir.dt.float32
BF16 = mybir.dt.bfloat16

WARMUP_MMS = int(os.environ.get("KERNEL_WARMUP", "12"))


@with_exitstack
def _core_kernel(ctx: ExitStack, tc: tile.TileContext,
                 domt_d, reft_d, ref_d, wt_d, bias_d, out_d):
    nc = tc.nc

    consts = ctx.enter_context(tc.tile_pool(name="consts", bufs=1))

    ps_S = ctx.enter_context(tc.tile_pool(name="ps_s", bufs=3, space="PSUM"))
    ps_X = ctx.enter_context(tc.tile_pool(name="ps_x", bufs=3, space="PSUM"))
    ps_R = ctx.enter_context(tc.tile_pool(name="ps_r", bufs=2, space="PSUM"))

    # PE warmup: dense dependency-free matmuls on memset zeros while the
    # first input DMAs stream, so the HAM clock gate reaches 8/8 before
    # real work arrives.
    zsrc = consts.tile([P, 640], BF16)
    nc.vector.memset(zsrc[:], 0.0)
    if WARMUP_MMS:
        warm_ps = ps_S.tile([P, 512], F32, tag="ps_s")
        for _ in range(WARMUP_MMS):
            nc.tensor.matmul(warm_ps[:], zsrc[:, :P], zsrc[:, P:640],
                             start=True, stop=True)

    ones_sb = consts.tile([P, 1], BF16)
    nc.vector.memset(ones_sb[:], 1.0)

    p_domT = ctx.enter_context(tc.tile_pool(name="domT", bufs=2))
    p_refT = ctx.enter_context(tc.tile_pool(name="refT", bufs=2))
    p_ref = ctx.enter_context(tc.tile_pool(name="ref", bufs=2))
    p_Pt = ctx.enter_context(tc.tile_pool(name="pt", bufs=2))
    p_x = ctx.enter_context(tc.tile_pool(name="x", bufs=2))
    p_out = ctx.enter_context(tc.tile_pool(name="out", bufs=4))
    p_stats = ctx.enter_context(tc.tile_pool(name="stats", bufs=8))

    # ---- input DMAs, most-urgent first, alternating the two input rings
    def load_T_half(sb, dr, b, h, eng):
        # [C, N] tensor, n-half h: k-chunk k lands at sb cols
        # [k*N + h*512, k*N + (h+1)*512)
        eng.dma_start(
            sb.rearrange("p (k n) -> p k n", k=KC)[:, :, h * 512:(h + 1) * 512],
            dr[b].rearrange("(k p) n -> p k n", p=P)[:, :, h * 512:(h + 1) * 512],
        )

    def load_T_mi(sb, dr, b, mi, eng):
        # [C, N] tensor, single m-tile: cols [mi*P, (mi+1)*P) of each chunk
        eng.dma_start(
            sb.rearrange("p (k n) -> p k n", k=KC)[:, :, mi * P:(mi + 1) * P],
            dr[b].rearrange("(k p) n -> p k n", p=P)[:, :, mi * P:(mi + 1) * P],
        )

    def load_T_full(sb, dr, b, eng):
        eng.dma_start(
            sb.rearrange("p (k n) -> p k n", k=KC),
            dr[b].rearrange("(k p) n -> p k n", p=P),
        )

    def load_nat(sb, dr, b, eng):
        # tile col block t holds rows [128t, 128(t+1)) of the [N, C] matrix
        eng.dma_start(
            sb.rearrange("p (t c) -> p t c", t=NT),
            dr[b].rearrange("(t p) c -> p t c", p=P),
        )

    domT_sbs = [p_domT.tile([P, KC * N], BF16, tag="domT", name=f"domT_sb{i}")
                for i in range(BPC)]
    refT_sbs = [p_refT.tile([P, KC * N], BF16, tag="refT", name=f"refT_sb{i}")
                for i in range(BPC)]
    ref_sbs = [p_ref.tile([P, NT * C], BF16, tag="ref", name=f"ref_sb{i}")
               for i in range(BPC)]
    wt_sb = consts.tile([P, KC * C], BF16)
    bias_sb = consts.tile([P, C], F32)

    # ring A (SWDGE/gpsimd): domT b0 halves, ref b0, then batch 1
    # ring B (SP HWDGE/sync): refT b0 per-m-tile (fine-grained unlock),
    #                         wt, bias, then batch 1
    load_T_half(domT_sbs[0], domt_d, 0, 0, nc.gpsimd)
    for mi in range(MT):
        load_T_mi(refT_sbs[0], reft_d, 0, mi, nc.sync)
    load_T_half(domT_sbs[0], domt_d, 0, 1, nc.gpsimd)
    load_nat(ref_sbs[0], ref_d, 0, nc.gpsimd)
    nc.sync.dma_start(wt_sb.rearrange("p (q c) -> p q c", q=KC),
                      wt_d.rearrange("(q p) c -> p q c", p=P))
    nc.sync.dma_start(bias_sb[:], bias_d.partition_broadcast(P))
    if BPC > 1:
        load_T_full(domT_sbs[1], domt_d, 1, nc.gpsimd)
        load_T_full(refT_sbs[1], reft_d, 1, nc.sync)
        load_nat(ref_sbs[1], ref_d, 1, nc.gpsimd)

    def stage1(b):
        # S^T = ref @ dom^T (chunked over c); P^T = exp(S^T * SCALE)
        Pt_tiles = [p_Pt.tile([P, N], BF16, tag=f"pt{mi}", name=f"pt{b}_{mi}")
                    for mi in range(MT)]
        for h in range(MH):
            for mi in range(MT):
                ps_s = ps_S.tile([P, 512], F32, tag="ps_s",
                                 name=f"ps_s{b}_{h}_{mi}")
                for k in range(KC):
                    nc.tensor.matmul(
                        ps_s[:],
                        refT_sbs[b][:, k * N + mi * P: k * N + (mi + 1) * P],
                        domT_sbs[b][:, k * N + h * 512: k * N + (h + 1) * 512],
                        start=(k == 0), stop=(k == KC - 1),
                    )
                nc.scalar.activation(Pt_tiles[mi][:, h * 512:(h + 1) * 512],
                                     ps_s[:],
                                     mybir.ActivationFunctionType.Exp,
                                     scale=float(SCALE))
        return Pt_tiles

    def make_emit_half(b, x_tiles):
        out_v = out_d[b].rearrange("(n2 two) j -> two n2 j", two=2)

        def emit_half_out(e):
            # out rows (2*cp + e) = x_half_e^T @ proj_w^T + bias
            for cb in range(JT):
                ps_z = ps_X.tile([P, C], F32, tag="ps_x",
                                 name=f"ps_z{b}_{e}_{cb}")
                for q in range(KC):
                    x_t = x_tiles[e * KC + q]
                    nc.tensor.matmul(
                        ps_z[:],
                        x_t[:, cb * P:(cb + 1) * P],
                        wt_sb[:, q * C:(q + 1) * C],
                        start=(q == 0), stop=(q == KC - 1),
                    )
                o_sb = p_out.tile([P, C], F32, tag="out", name=f"o{b}_{e}_{cb}")
                nc.vector.tensor_add(o_sb[:], ps_z[:], bias_sb[:])
                nc.scalar.dma_start(out_v[e, cb * P:(cb + 1) * P, :], o_sb[:])

        return emit_half_out

    def stage2(b, Pt_tiles, x_tiles, emit_half_out):
        # x = P @ ref, row-sums via rank-1 ones matmuls, normalize at evict
        for nt in range(NT):
            ps_x = ps_X.tile([P, C], F32, tag="ps_x", name=f"ps_x{b}_{nt}")
            ps_r = ps_R.tile([P, 1], F32, tag="ps_r", name=f"ps_r{b}_{nt}")
            for mi in range(MT):
                lhsT = Pt_tiles[mi][:, nt * P:(nt + 1) * P]
                nc.tensor.matmul(ps_x[:], lhsT,
                                 ref_sbs[b][:, mi * C:(mi + 1) * C],
                                 start=(mi == 0), stop=(mi == MT - 1))
                nc.tensor.matmul(ps_r[:], lhsT, ones_sb[:],
                                 start=(mi == 0), stop=(mi == MT - 1))
            recip = p_stats.tile([P, 1], F32, tag="recip", name=f"rc{b}_{nt}")
            nc.vector.reciprocal(recip[:], ps_r[:])
            x_t = p_x.tile([P, C], BF16, tag=f"x{nt}", name=f"x{b}_{nt}")
            nc.scalar.mul(x_t[:], ps_x[:], recip[:])
            x_tiles.append(x_t)
            if nt == KC:
                # half 0's x tiles done one group ago — the gap hides the
                # x-evict latency behind this nt's matmuls
                emit_half_out(0)

    # batch interleave: b0 s1, b0 s2(+proj half0), b1 s1, b0 proj half1,
    # b1 s2(+proj half0), b1 proj half1 — so the proj of a finished half
    # always has preceding PE work covering the x-evict latency.
    xs, emits = {}, {}
    Pts = {}
    Pts[0] = stage1(0)
    xs[0] = []
    emits[0] = make_emit_half(0, xs[0])
    stage2(0, Pts[0], xs[0], emits[0])
    if BPC > 1:
        Pts[1] = stage1(1)
    emits[0](1)
    if BPC > 1:
        xs[1] = []
        emits[1] = make_emit_half(1, xs[1])
        stage2(1, Pts[1], xs[1], emits[1])
        emits[1](1)


_CACHED = {}


def _build():
    key = ("nc", WARMUP_MMS)
    if key in _CACHED:
        return _CACHED[key]
    nc = bacc.Bacc("TRN2", target_bir_lowering=False, debug=False)
    domt_d = nc.dram_tensor("domt", [BPC, C, N], BF16, kind="ExternalInput").ap()
    reft_d = nc.dram_tensor("reft", [BPC, C, N], BF16, kind="ExternalInput").ap()
    ref_d = nc.dram_tensor("ref", [BPC, N, C], BF16, kind="ExternalInput").ap()
    wt_d = nc.dram_tensor("wt", [C, C], BF16, kind="ExternalInput").ap()
    bias_d = nc.dram_tensor("bias", [C], F32, kind="ExternalInput").ap()
    out_d = nc.dram_tensor("out", [BPC, N, C], F32, kind="ExternalOutput").ap()

    with tile.TileContext(nc) as tc:
        _core_kernel(tc, domt_d, reft_d, ref_d, wt_d, bias_d, out_d)
    nc.compile()
    _CACHED[key] = nc
    return nc


LAST_RESULTS = None


def kernel(dom, ref, proj_w, proj_b):
    global LAST_RESULTS
    bf16 = ml_dtypes.bfloat16
    dom = np.asarray(dom, dtype=np.float32)
    ref = np.asarray(ref, dtype=np.float32)
    wt = np.ascontiguousarray(np.asarray(proj_w, dtype=np.float32).T.astype(bf16))
    bias = np.ascontiguousarray(np.asarray(proj_b, dtype=np.float32))

    domt = np.ascontiguousarray(dom.transpose(0, 2, 1).astype(bf16))
    reft = np.ascontiguousarray(ref.transpose(0, 2, 1).astype(bf16))
    refn = np.ascontiguousarray(ref.astype(bf16))
    nc = _build()
    in_maps = [
        {
            "domt": domt[c * BPC:(c + 1) * BPC],
            "reft": reft[c * BPC:(c + 1) * BPC],
            "ref": refn[c * BPC:(c + 1) * BPC],
            "wt": wt,
            "bias": bias,
        }
        for c in range(CORES)
    ]
    res = run_bass_kernel_spmd(nc, in_maps, list(range(CORES)))
    LAST_RESULTS = res
    if res.exec_time_ns is not None:
        print(f"HW exec time: {res.exec_time_ns} ns")
    return np.concatenate([r["out"] for r in res.results], axis=0)
